# revision 5
# baseline (speedup 1.0000x reference)
"""Multi-head attention (B=8, S=2048, D=1024, H=16, DK=64) on 8 TRN2 NeuronCores.

Sharding: pure batch data-parallel — core i computes batch i's full attention.
No collectives needed; per-core output is the final [S, D] slice.

Per-core pipeline (all matmuls bf16, fp32 PSUM accumulation):
  1. gpsimd cast-DMA inputs f32->bf16 into DRAM staging, then HW DMA-transpose
     loads to get qT/kT/vT in [D, S] SBUF layout (contraction dim on partitions).
  2. Projections with head-PAIR packed weights: lhsT = [d, 2*64] so one matmul
     yields two heads' projected rows. q/k projected transposed [dk, s]; v
     projected natural [t, dk] with a ones column appended (softmax denominator
     comes out of the attention*V matmul for free).
  3. Scores computed transposed: scoresT[t, s] = kT_h.T @ qT_h, two heads
     row-packed into array rows 0-63 / 64-127 (K=64 each, concurrent).
  4. exp((1/32)*x) fused on ScalarE reading PSUM [128, 1024], writing bf16.
  5. AV: lhsT = [v_h | ones] [t, 65] -> out rows 0-63 = out_hT, row 64 = denom.
  6. normalize: reciprocal(denom) -> broadcast via K=1 outer-product matmul ->
     multiply; final Wo projection from transposed out tiles.
"""

import sys

if "/opt/trn_rl_repo" not in sys.path:
    sys.path.insert(0, "/opt/trn_rl_repo")

import functools
from contextlib import ExitStack

import numpy as np

import concourse.bass as bass
import concourse.mybir as mybir
import concourse.tile as tile
from concourse import bacc
from concourse.bass_utils import run_bass_kernel_spmd

F32 = mybir.dt.float32
BF16 = mybir.dt.bfloat16
P = 128

B, D, H, DK = 8, 1024, 16, 64
S_FULL = 2048
NPAIR = H // 2  # 8 head pairs
DT = D // P  # 8 d-tiles (contraction tiles for projections)
N_CORES = 8


def _body(ctx: ExitStack, tc: tile.TileContext, S: int):
    nc = tc.nc
    TT = S // P  # t-tiles
    SCW = min(1024, S)  # attention s-chunk width
    SC = S // SCW  # number of s chunks
    W5 = min(512, S)  # matmul free-dim width (one PSUM bank)
    NH = SCW // W5  # W5-wide halves per chunk

    q_ap = nc.dram_tensor("q", [S, D], F32, kind="ExternalInput").ap()
    k_ap = nc.dram_tensor("k", [S, D], F32, kind="ExternalInput").ap()
    v_ap = nc.dram_tensor("v", [S, D], F32, kind="ExternalInput").ap()
    wq_ap = nc.dram_tensor("Wq", [H, D, DK], F32, kind="ExternalInput").ap()
    wk_ap = nc.dram_tensor("Wk", [H, D, DK], F32, kind="ExternalInput").ap()
    wv_ap = nc.dram_tensor("Wv", [H, D, DK], F32, kind="ExternalInput").ap()
    wo_ap = nc.dram_tensor("Wo", [D, D], F32, kind="ExternalInput").ap()
    out_ap = nc.dram_tensor("out", [S, D], F32, kind="ExternalOutput").ap()

    scale = float(D) ** -0.5

    dram = ctx.enter_context(tc.tile_pool(name="dram", bufs=1, space="DRAM"))
    consts = ctx.enter_context(tc.tile_pool(name="consts", bufs=1))
    wpool = ctx.enter_context(tc.tile_pool(name="wpool", bufs=2))
    res = ctx.enter_context(tc.tile_pool(name="res", bufs=1))
    xpool = ctx.enter_context(tc.tile_pool(name="xpool", bufs=1))
    apool = ctx.enter_context(tc.tile_pool(name="apool", bufs=3))
    spool = ctx.enter_context(tc.tile_pool(name="spool", bufs=1))
    fpool = ctx.enter_context(tc.tile_pool(name="fpool", bufs=2))
    ps_sc = ctx.enter_context(tc.tile_pool(name="ps_sc", bufs=2, space="PSUM"))
    ps_av = ctx.enter_context(tc.tile_pool(name="ps_av", bufs=2, space="PSUM"))

    ones_sb = consts.tile([1, DK], BF16, tag="ones")
    nc.vector.memset(ones_sb[:], 1.0)

    # ---- weights ----
    # pair-packed projection weights: w[p, dt, pair, h2, dk] (bf16, cast in DMA)
    w_tiles = {}
    for name, wap in (("wk", wk_ap), ("wv", wv_ap), ("wq", wq_ap)):
        wt = wpool.tile([P, DT, NPAIR, 2, DK], BF16, tag="w", name=name)
        src = wap.rearrange("h (dt p) k -> p dt h k", p=P)
        for dt_ in range(DT):
            nc.gpsimd.dma_start(wt[:, dt_], src[:, dt_])
        w_tiles[name] = wt

    # WoT[p, kt, dout] = Wo[dout, kt*128 + p] via bf16 staging + DMA transpose
    wo_stage = dram.tile([D, D], BF16, tag="wo_stage")
    nc.gpsimd.dma_start(wo_stage[:], wo_ap)
    woT = res.tile([P, DT, D], BF16, tag="woT")
    for kt in range(DT):
        nc.sync.dma_start_transpose(woT[:, kt, :], wo_stage[:, kt * P : (kt + 1) * P])

    # ---- inputs: stage bf16, transpose-load to [d, s] layout ----
    def load_xT(x_ap, label):
        stage = dram.tile([S, D], BF16, tag=f"stage_{label}")
        n_chunks = 4
        rows = S // n_chunks
        for c in range(n_chunks):
            sl = slice(c * rows, (c + 1) * rows)
            nc.gpsimd.dma_start(stage[sl, :], x_ap[sl, :])
        xT = xpool.tile([P, DT, S], BF16, tag="xT", name=f"{label}T")
        for dt_ in range(DT):
            nc.sync.dma_start_transpose(
                xT[:, dt_, :], stage[:, dt_ * P : (dt_ + 1) * P]
            )
        return xT

    # ---- projections ----
    kproj = res.tile([P, NPAIR, S], BF16, tag="kproj")
    qproj = res.tile([P, NPAIR, S], BF16, tag="qproj")
    vaug = res.tile([P, H, TT, DK + 1], BF16, tag="vaug")
    nc.vector.memset(vaug[:, :, :, DK : DK + 1], 1.0)

    def project_T(xT, w, dst):
        # dst[h2*64+dk, pair, s] = sum_d w[d, pair, h2, dk] * xT[d, s]
        for pr in range(NPAIR):
            for scq in range(S // W5):
                ps = ps_sc.tile([P, W5], F32, tag="sc", name="proj_ps")
                for dt_ in range(DT):
                    nc.tensor.matmul(
                        ps,
                        w[:, dt_, pr],
                        xT[:, dt_, scq * W5 : (scq + 1) * W5],
                        start=dt_ == 0,
                        stop=dt_ == DT - 1,
                    )
                nc.vector.tensor_copy(
                    out=dst[:, pr, scq * W5 : (scq + 1) * W5], in_=ps
                )

    kT = load_xT(k_ap, "k")
    project_T(kT, w_tiles["wk"], kproj)

    vT = load_xT(v_ap, "v")
    for tt in range(TT):
        for half in range(2):
            ps = ps_sc.tile([P, 512], F32, tag="sc", name="vproj_ps")
            for dt_ in range(DT):
                nc.tensor.matmul(
                    ps,
                    vT[:, dt_, tt * P : (tt + 1) * P],
                    w_tiles["wv"][:, dt_, half * 4 : (half + 1) * 4],
                    start=dt_ == 0,
                    stop=dt_ == DT - 1,
                )
            nc.vector.tensor_copy(
                out=vaug[:, half * 8 : (half + 1) * 8, tt, 0:DK],
                in_=ps.rearrange("p (h k) -> p h k", k=DK),
            )

    qT = load_xT(q_ap, "q")
    project_T(qT, w_tiles["wq"], qproj)

    # ---- attention + output projection, per s-chunk ----
    exp_f = mybir.ActivationFunctionType.Exp
    for sc_ in range(SC):
        s0 = sc_ * SCW
        outT = wpool.tile([P, NPAIR, SCW], BF16, tag="w", name="outT")
        for pr in range(NPAIR):
            av_ps = [
                ps_av.tile([DK + 1, SCW], F32, tag="av", name=f"av{h2}")
                for h2 in range(2)
            ]
            for tt in range(TT):
                sc_ps = [
                    ps_sc.tile([P, SCW], F32, tag="sc", name=f"sc{h2}")
                    for h2 in range(2)
                ]
                for h2 in range(2):
                    rows = slice(h2 * DK, (h2 + 1) * DK)
                    lhsT = kproj[rows, pr, tt * P : (tt + 1) * P]
                    for sh in range(NH):
                        nc.tensor.matmul(
                            sc_ps[h2][:, sh * W5 : (sh + 1) * W5],
                            lhsT,
                            qproj[rows, pr, s0 + sh * W5 : s0 + (sh + 1) * W5],
                        )
                ats = []
                for h2 in range(2):
                    at = apool.tile([P, SCW], BF16, tag="attn", name="at")
                    nc.scalar.activation(at[:], sc_ps[h2][:], exp_f, scale=scale)
                    ats.append(at)
                for h2 in range(2):
                    va = vaug[:, 2 * pr + h2, tt, :]
                    for sh in range(NH):
                        nc.tensor.matmul(
                            av_ps[h2][:, sh * W5 : (sh + 1) * W5],
                            va,
                            ats[h2][:, sh * W5 : (sh + 1) * W5],
                            start=tt == 0,
                            stop=tt == TT - 1,
                        )
            # normalize: out_hT = av[0:64] * (1 / av[64])  broadcast over rows
            for h2 in range(2):
                rec = spool.tile([1, SCW], F32, tag="rec")
                nc.vector.reciprocal(rec[:], av_ps[h2][DK : DK + 1, :])
                recb = spool.tile([1, SCW], BF16, tag="recb")
                nc.vector.tensor_copy(out=recb[:], in_=rec[:])
                bc_ps = ps_sc.tile([DK, SCW], F32, tag="sc", name="bc_ps")
                for sh in range(NH):
                    nc.tensor.matmul(
                        bc_ps[:, sh * W5 : (sh + 1) * W5],
                        ones_sb[:],
                        recb[:, sh * W5 : (sh + 1) * W5],
                    )
                bc_sb = spool.tile([DK, SCW], BF16, tag="bc_sb")
                nc.vector.tensor_copy(out=bc_sb[:], in_=bc_ps[:])
                nc.vector.tensor_tensor(
                    outT[h2 * DK : (h2 + 1) * DK, pr, :],
                    av_ps[h2][0:DK, :],
                    bc_sb[:],
                    mybir.AluOpType.mult,
                )
        # final projection for this s chunk
        for st in range(SCW // P):
            for dc in range(D // 512):
                f_ps = ps_sc.tile([P, 512], F32, tag="sc", name="f_ps")
                for kt in range(DT):
                    nc.tensor.matmul(
                        f_ps,
                        outT[:, kt, st * P : (st + 1) * P],
                        woT[:, kt, dc * 512 : (dc + 1) * 512],
                        start=kt == 0,
                        stop=kt == DT - 1,
                    )
                fo = fpool.tile([P, 512], F32, tag="fo")
                nc.vector.tensor_copy(out=fo[:], in_=f_ps[:])
                nc.sync.dma_start(
                    out_ap[s0 + st * P : s0 + (st + 1) * P, dc * 512 : (dc + 1) * 512],
                    fo[:],
                )


@functools.lru_cache(maxsize=2)
def build(S: int = S_FULL):
    nc = bacc.Bacc("TRN2", target_bir_lowering=False, debug=False)
    with tile.TileContext(nc) as tc:
        with ExitStack() as ctx:
            _body(ctx, tc, S)
    nc.compile()
    return nc


def kernel(**inputs: np.ndarray) -> np.ndarray:
    query = np.ascontiguousarray(inputs["query"], dtype=np.float32)
    key = np.ascontiguousarray(inputs["key"], dtype=np.float32)
    value = np.ascontiguousarray(inputs["value"], dtype=np.float32)
    Wq = np.ascontiguousarray(inputs["Wq"], dtype=np.float32)
    Wk = np.ascontiguousarray(inputs["Wk"], dtype=np.float32)
    Wv = np.ascontiguousarray(inputs["Wv"], dtype=np.float32)
    Wo = np.ascontiguousarray(inputs["Wo"], dtype=np.float32)

    nc = build(S_FULL)
    in_maps = [
        {
            "q": query[i],
            "k": key[i],
            "v": value[i],
            "Wq": Wq,
            "Wk": Wk,
            "Wv": Wv,
            "Wo": Wo,
        }
        for i in range(N_CORES)
    ]
    res = run_bass_kernel_spmd(nc, in_maps, core_ids=list(range(N_CORES)))
    return np.stack([res.results[i]["out"] for i in range(N_CORES)], axis=0)


if __name__ == "__main__":
    rng = np.random.default_rng(0)
    ins = {
        "query": rng.standard_normal((B, S_FULL, D), dtype=np.float32),
        "key": rng.standard_normal((B, S_FULL, D), dtype=np.float32),
        "value": rng.standard_normal((B, S_FULL, D), dtype=np.float32),
        "Wq": rng.standard_normal((H, D, DK), dtype=np.float32) * 0.02,
        "Wk": rng.standard_normal((H, D, DK), dtype=np.float32) * 0.02,
        "Wv": rng.standard_normal((H, D, DK), dtype=np.float32) * 0.02,
        "Wo": rng.standard_normal((D, D), dtype=np.float32) * 0.02,
    }
    out = kernel(**ins)
    print(out.shape, out.dtype)


# revision 10
# speedup vs baseline: 15649.6939x; 15649.6939x over previous
"""Multi-head attention (B=8, S=2048, D=1024, H=16, DK=64) on 8 TRN2 NeuronCores.

Sharding: pure batch data-parallel — core i computes batch i's full attention.
No collectives needed; per-core output is the final [S, D] slice.

Per-core pipeline (all matmuls bf16, fp32 PSUM accumulation):
  1. gpsimd cast-DMA inputs f32->bf16 into DRAM staging, then HW DMA-transpose
     loads to get qT/kT/vT in [D, S] SBUF layout (contraction dim on partitions).
  2. Projections with head-PAIR packed weights: lhsT = [d, 2*64] so one matmul
     yields two heads' projected rows. q/k projected transposed [dk, s]; v
     projected natural [t, dk] with a ones column appended (softmax denominator
     comes out of the attention*V matmul for free).
  3. Scores computed transposed: scoresT[t, s] = kT_h.T @ qT_h, two heads
     row-packed into array rows 0-63 / 64-127 (K=64 each, concurrent).
  4. exp((1/32)*x) fused on ScalarE reading PSUM [128, 1024], writing bf16.
  5. AV: lhsT = [v_h | ones] [t, 65] -> out rows 0-63 = out_hT, row 64 = denom.
  6. normalize: reciprocal(denom) -> gpsimd partition_broadcast -> multiply;
     final Wo projection from transposed out tiles, interleaved with the next
     s-chunk's attention to keep ScalarE fed.
"""

import sys

if "/opt/trn_rl_repo" not in sys.path:
    sys.path.insert(0, "/opt/trn_rl_repo")

import functools
from contextlib import ExitStack

import numpy as np

import concourse.bass as bass
import concourse.mybir as mybir
import concourse.tile as tile
from concourse import bacc
from concourse.bass_utils import run_bass_kernel_spmd

F32 = mybir.dt.float32
BF16 = mybir.dt.bfloat16
P = 128

B, D, H, DK = 8, 1024, 16, 64
S_FULL = 2048
NPAIR = H // 2  # 8 head pairs
DT = D // P  # 8 d-tiles (contraction tiles for projections)
N_CORES = 8


def _body(ctx: ExitStack, tc: tile.TileContext, S: int):
    nc = tc.nc
    TT = S // P  # t-tiles
    SCW = min(1024, S)  # attention s-chunk width
    SC = S // SCW  # number of s chunks
    W5 = min(512, S)  # matmul free-dim width (one PSUM bank)
    NH = SCW // W5  # W5-wide halves per chunk

    q_ap = nc.dram_tensor("q", [S, D], F32, kind="ExternalInput").ap()
    k_ap = nc.dram_tensor("k", [S, D], F32, kind="ExternalInput").ap()
    v_ap = nc.dram_tensor("v", [S, D], F32, kind="ExternalInput").ap()
    wq_ap = nc.dram_tensor("Wq", [H, D, DK], F32, kind="ExternalInput").ap()
    wk_ap = nc.dram_tensor("Wk", [H, D, DK], F32, kind="ExternalInput").ap()
    wv_ap = nc.dram_tensor("Wv", [H, D, DK], F32, kind="ExternalInput").ap()
    wo_ap = nc.dram_tensor("Wo", [D, D], F32, kind="ExternalInput").ap()
    out_ap = nc.dram_tensor("out", [S, D], F32, kind="ExternalOutput").ap()

    scale = float(D) ** -0.5

    dram = ctx.enter_context(tc.tile_pool(name="dram", bufs=1, space="DRAM"))
    consts = ctx.enter_context(tc.tile_pool(name="consts", bufs=1))
    wpool = ctx.enter_context(tc.tile_pool(name="wpool", bufs=2))
    res = ctx.enter_context(tc.tile_pool(name="res", bufs=1))
    # PSUM: "sc" = attention scores (2 x 2 banks); "av" = AV accumulators,
    # projections and the final Wo projection share it (2 x 2 banks).
    ps_sc = ctx.enter_context(tc.tile_pool(name="ps_sc", bufs=2, space="PSUM"))
    ps_av = ctx.enter_context(tc.tile_pool(name="ps_av", bufs=2, space="PSUM"))

    # ---- weights: pair-packed w[p, dt, pair, h2, dk] (bf16, cast in DMA);
    # loads are emitted just before their consumer so the gpsimd DMA queue
    # never delays the k staging chain ----
    def load_w(name, wap):
        wt = wpool.tile([P, DT, NPAIR, 2, DK], BF16, tag="w", name=name)
        srcw = wap.rearrange("h (dt p) k -> p dt h k", p=P)
        for dt_ in range(DT):
            nc.gpsimd.dma_start(wt[:, dt_], srcw[:, dt_])
        return wt

    # ---- inputs: stage bf16, transpose-load to [d, s] layout ----
    # chunked so each transpose only waits for its own staging rows
    def load_xT(xpool, x_ap, label):
        stage = dram.tile([S, D], BF16, tag=f"stage_{label}")
        n_chunks = max(1, S // 512)
        rows = S // n_chunks
        xT = xpool.tile([P, DT, S], BF16, tag="xT", name=f"{label}T")
        for c in range(n_chunks):
            sl = slice(c * rows, (c + 1) * rows)
            nc.gpsimd.dma_start(stage[sl, :], x_ap[sl, :])
            for dt_ in range(DT):
                nc.sync.dma_start_transpose(
                    xT[:, dt_, sl], stage[sl, dt_ * P : (dt_ + 1) * P]
                )
        return xT

    # ---- projections (PSUM tiles on the "av" tag so the attention-score
    # pipeline's "sc" slots are free from the start) ----
    woT = res.tile([P, DT, D], BF16, tag="woT")
    kproj = res.tile([P, NPAIR, S], BF16, tag="kproj")
    qproj = res.tile([P, NPAIR, S], BF16, tag="qproj")
    vaug = res.tile([P, H, TT, DK + 1], BF16, tag="vaug")
    nc.vector.memset(vaug[:, :, :, DK : DK + 1], 1.0)

    ones_sb = consts.tile([1, DK], BF16, tag="ones")
    nc.vector.memset(ones_sb[:], 1.0)

    def project_T(xT, w, dst):
        # dst[h2*64+dk, pair, s] = sum_d w[d, pair, h2, dk] * xT[d, s]
        for pr in range(NPAIR):
            for scq in range(S // W5):
                ps = ps_av.tile([P, W5], F32, tag="av", name="proj_ps")
                for dt_ in range(DT):
                    nc.tensor.matmul(
                        ps,
                        w[:, dt_, pr],
                        xT[:, dt_, scq * W5 : (scq + 1) * W5],
                        start=dt_ == 0,
                        stop=dt_ == DT - 1,
                    )
                nc.vector.tensor_copy(
                    out=dst[:, pr, scq * W5 : (scq + 1) * W5], in_=ps
                )

    with tc.tile_pool(name="xpool", bufs=1) as xpool:
        wk = load_w("wk", wk_ap)
        kT = load_xT(xpool, k_ap, "k")
        project_T(kT, wk, kproj)
        wv = load_w("wv", wv_ap)

        # v projected natural [t, h*dk]; half-major so heads 0-7 finish first
        vT = load_xT(xpool, v_ap, "v")
        for half in range(2):
            for tt in range(TT):
                ps = ps_av.tile([P, 512], F32, tag="av", name="vproj_ps")
                for dt_ in range(DT):
                    nc.tensor.matmul(
                        ps,
                        vT[:, dt_, tt * P : (tt + 1) * P],
                        wv[:, dt_, half * 4 : (half + 1) * 4],
                        start=dt_ == 0,
                        stop=dt_ == DT - 1,
                    )
                nc.vector.tensor_copy(
                    out=vaug[:, half * 8 : (half + 1) * 8, tt, 0:DK],
                    in_=ps.rearrange("p (h k) -> p h k", k=DK),
                )

        # q last, pair-major: attention for pair 0 unblocks as soon as its
        # q slice is projected, overlapping the rest of q-proj with attention
        wq = load_w("wq", wq_ap)
        qT = load_xT(xpool, q_ap, "q")
        project_T(qT, wq, qproj)

        # Wo transpose staging, needed only once the first s-chunk finishes
        wo_stage = dram.tile([D, D], BF16, tag="wo_stage")
        nc.gpsimd.dma_start(wo_stage[:], wo_ap)
        for kt in range(DT):
            nc.sync.dma_start_transpose(
                woT[:, kt, :], wo_stage[:, kt * P : (kt + 1) * P]
            )

    # xpool released: its 32KB/partition becomes the deep attn-tile queue,
    # letting ScalarE run many t-tiles ahead of the AV matmuls
    apool = ctx.enter_context(tc.tile_pool(name="apool", bufs=14))
    spool = ctx.enter_context(tc.tile_pool(name="spool", bufs=1))
    fpool = ctx.enter_context(tc.tile_pool(name="fpool", bufs=3))

    # ---- attention; the previous chunk's Wo projection is interleaved into
    # the pair loop so its PSUM/PE use rides along without starving ScalarE ----
    exp_f = mybir.ActivationFunctionType.Exp

    def final_proj_step(outT_prev, sc_prev, st):
        s0p = sc_prev * SCW
        for dc in range(D // W5):
            f_ps = ps_av.tile([P, W5], F32, tag="av", name="f_ps")
            for kt in range(DT):
                nc.tensor.matmul(
                    f_ps,
                    outT_prev[:, kt, st * P : (st + 1) * P],
                    woT[:, kt, dc * W5 : (dc + 1) * W5],
                    start=kt == 0,
                    stop=kt == DT - 1,
                )
            fo = fpool.tile([P, W5], F32, tag="fo")
            nc.vector.tensor_copy(out=fo[:], in_=f_ps[:])
            nc.sync.dma_start(
                out_ap[s0p + st * P : s0p + (st + 1) * P, dc * W5 : (dc + 1) * W5],
                fo[:],
            )

    outT_prev = None
    for sc_ in range(SC):
        s0 = sc_ * SCW
        outT = wpool.tile([P, NPAIR, SCW], BF16, tag="w", name="outT")
        st_per_pair = max(1, (SCW // P) // NPAIR)  # final-proj subtiles per pair
        for pr in range(NPAIR):
            av_ps = [
                ps_av.tile([DK + 1, SCW], F32, tag="av", name=f"av{h2}")
                for h2 in range(2)
            ]

            def emit_scores(tt):
                sc_ps = [
                    ps_sc.tile([P, SCW], F32, tag="sc", name=f"sc{h2}")
                    for h2 in range(2)
                ]
                for h2 in range(2):
                    rows = slice(h2 * DK, (h2 + 1) * DK)
                    lhsT = kproj[rows, pr, tt * P : (tt + 1) * P]
                    for sh in range(NH):
                        nc.tensor.matmul(
                            sc_ps[h2][:, sh * W5 : (sh + 1) * W5],
                            lhsT,
                            qproj[rows, pr, s0 + sh * W5 : s0 + (sh + 1) * W5],
                        )
                return sc_ps

            # software-pipelined: scores(tt+1) is emitted (and thus scheduled
            # on the PE) before AV(tt), so ScalarE's next input is never
            # queued behind the AV matmuls
            sc_ps = emit_scores(0)
            for tt in range(TT):
                ats = []
                for h2 in range(2):
                    at = apool.tile([P, SCW], BF16, tag="attn", name="at")
                    nc.scalar.activation(at[:], sc_ps[h2][:], exp_f, scale=scale)
                    ats.append(at)
                if tt + 1 < TT:
                    sc_ps = emit_scores(tt + 1)
                for h2 in range(2):
                    va = vaug[:, 2 * pr + h2, tt, :]
                    for sh in range(NH):
                        nc.tensor.matmul(
                            av_ps[h2][:, sh * W5 : (sh + 1) * W5],
                            va,
                            ats[h2][:, sh * W5 : (sh + 1) * W5],
                            start=tt == 0,
                            stop=tt == TT - 1,
                        )
            # normalize: out_hT = av[0:64] * (1 / av[64]) broadcast over rows
            for h2 in range(2):
                rec = spool.tile([1, SCW], F32, tag="rec")
                nc.vector.reciprocal(rec[:], av_ps[h2][DK : DK + 1, :])
                recb = spool.tile([1, SCW], BF16, tag="recb")
                nc.vector.tensor_copy(out=recb[:], in_=rec[:])
                bc_sb = spool.tile([DK, SCW], BF16, tag="bc_sb")
                nc.gpsimd.partition_broadcast(bc_sb[:], recb[:])
                nc.vector.tensor_tensor(
                    outT[h2 * DK : (h2 + 1) * DK, pr, :],
                    av_ps[h2][0:DK, :],
                    bc_sb[:],
                    mybir.AluOpType.mult,
                )
            # weave the previous chunk's output projection into this pair loop
            if outT_prev is not None:
                for i in range(st_per_pair):
                    st = pr * st_per_pair + i
                    if st < SCW // P:
                        final_proj_step(outT_prev, sc_ - 1, st)
        outT_prev = outT

    for st in range(SCW // P):
        final_proj_step(outT_prev, SC - 1, st)


@functools.lru_cache(maxsize=2)
def build(S: int = S_FULL):
    nc = bacc.Bacc("TRN2", target_bir_lowering=False, debug=False)
    with tile.TileContext(nc) as tc:
        with ExitStack() as ctx:
            _body(ctx, tc, S)
    nc.compile()
    return nc


def kernel(**inputs: np.ndarray) -> np.ndarray:
    query = np.ascontiguousarray(inputs["query"], dtype=np.float32)
    key = np.ascontiguousarray(inputs["key"], dtype=np.float32)
    value = np.ascontiguousarray(inputs["value"], dtype=np.float32)
    Wq = np.ascontiguousarray(inputs["Wq"], dtype=np.float32)
    Wk = np.ascontiguousarray(inputs["Wk"], dtype=np.float32)
    Wv = np.ascontiguousarray(inputs["Wv"], dtype=np.float32)
    Wo = np.ascontiguousarray(inputs["Wo"], dtype=np.float32)

    nc = build(S_FULL)
    in_maps = [
        {
            "q": query[i],
            "k": key[i],
            "v": value[i],
            "Wq": Wq,
            "Wk": Wk,
            "Wv": Wv,
            "Wo": Wo,
        }
        for i in range(N_CORES)
    ]
    res = run_bass_kernel_spmd(nc, in_maps, core_ids=list(range(N_CORES)))
    return np.stack([res.results[i]["out"] for i in range(N_CORES)], axis=0)


if __name__ == "__main__":
    rng = np.random.default_rng(0)
    ins = {
        "query": rng.standard_normal((B, S_FULL, D), dtype=np.float32),
        "key": rng.standard_normal((B, S_FULL, D), dtype=np.float32),
        "value": rng.standard_normal((B, S_FULL, D), dtype=np.float32),
        "Wq": rng.standard_normal((H, D, DK), dtype=np.float32) * 0.02,
        "Wk": rng.standard_normal((H, D, DK), dtype=np.float32) * 0.02,
        "Wv": rng.standard_normal((H, D, DK), dtype=np.float32) * 0.02,
        "Wo": rng.standard_normal((D, D), dtype=np.float32) * 0.02,
    }
    out = kernel(**ins)
    print(out.shape, out.dtype)


# revision 22
# speedup vs baseline: 17398.9244x; 1.1118x over previous
"""Multi-head attention (B=8, S=2048, D=1024, H=16, DK=64) on 8 TRN2 NeuronCores.

Sharding: pure batch data-parallel — core i computes batch i's full attention.
No collectives needed; per-core output is the final [S, D] slice.

Per-core pipeline (all matmuls bf16, fp32 PSUM accumulation):
  1. gpsimd cast-DMA inputs f32->bf16 into DRAM staging, then HW DMA-transpose
     loads to get qT/kT/vT in [D, S] SBUF layout (contraction dim on partitions).
  2. Projections with head-PAIR packed weights: lhsT = [d, 2*64] so one matmul
     yields two heads' projected rows. q/k projected transposed [dk, s]; v
     projected natural [t, dk] with a ones column appended (softmax denominator
     comes out of the attention*V matmul for free).
  3. Scores computed transposed: scoresT[t, s] = kT_h.T @ qT_h, two heads
     row-packed into array rows 0-63 / 64-127 (K=64 each, concurrent).
  4. exp((1/32)*x) fused on ScalarE reading PSUM [128, 1024], writing bf16.
  5. AV: lhsT = [v_h | ones] [t, 65] -> out rows 0-63 = out_hT, row 64 = denom.
  6. normalize: reciprocal(denom) -> gpsimd partition_broadcast -> multiply;
     final Wo projection from transposed out tiles, interleaved with the next
     s-chunk's attention to keep ScalarE fed.
"""

import sys

if "/opt/trn_rl_repo" not in sys.path:
    sys.path.insert(0, "/opt/trn_rl_repo")

import functools
from contextlib import ExitStack

import numpy as np

import concourse.bass as bass
import concourse.mybir as mybir
import concourse.tile as tile
from concourse import bacc
from concourse.bass_utils import run_bass_kernel_spmd
from concourse.tile_rust import add_dep_helper

F32 = mybir.dt.float32
BF16 = mybir.dt.bfloat16
P = 128

B, D, H, DK = 8, 1024, 16, 64
S_FULL = 2048
NPAIR = H // 2  # 8 head pairs
DT = D // P  # 8 d-tiles (contraction tiles for projections)
N_CORES = 8


def _body(ctx: ExitStack, tc: tile.TileContext, S: int):
    nc = tc.nc
    TT = S // P  # t-tiles
    SCW = min(1024, S)  # attention s-chunk width
    SC = S // SCW  # number of s chunks
    W5 = min(512, S)  # matmul free-dim width (one PSUM bank)
    NH = SCW // W5  # W5-wide halves per chunk

    q_ap = nc.dram_tensor("q", [S, D], F32, kind="ExternalInput").ap()
    k_ap = nc.dram_tensor("k", [S, D], F32, kind="ExternalInput").ap()
    v_ap = nc.dram_tensor("v", [S, D], F32, kind="ExternalInput").ap()
    wq_ap = nc.dram_tensor("Wq", [H, D, DK], F32, kind="ExternalInput").ap()
    wk_ap = nc.dram_tensor("Wk", [H, D, DK], F32, kind="ExternalInput").ap()
    wv_ap = nc.dram_tensor("Wv", [H, D, DK], F32, kind="ExternalInput").ap()
    wo_ap = nc.dram_tensor("Wo", [D, D], F32, kind="ExternalInput").ap()
    out_ap = nc.dram_tensor("out", [S, D], F32, kind="ExternalOutput").ap()

    scale = float(D) ** -0.5

    dram = ctx.enter_context(tc.tile_pool(name="dram", bufs=1, space="DRAM"))
    consts = ctx.enter_context(tc.tile_pool(name="consts", bufs=1))
    wpool = ctx.enter_context(tc.tile_pool(name="wpool", bufs=2))
    res = ctx.enter_context(tc.tile_pool(name="res", bufs=1))
    # PSUM: "sc" = attention scores (2 x 2 banks); "av" = AV accumulators,
    # projections and the final Wo projection share it (2 x 2 banks).
    ps_sc = ctx.enter_context(tc.tile_pool(name="ps_sc", bufs=2, space="PSUM"))
    ps_av = ctx.enter_context(tc.tile_pool(name="ps_av", bufs=2, space="PSUM"))

    # ---- weights: pair-packed w[p, dt, pair, h2, dk] (bf16, cast in DMA);
    # loads are emitted just before their consumer so the gpsimd DMA queue
    # never delays the k staging chain ----
    def load_w(name, wap):
        wt = wpool.tile([P, DT, NPAIR, 2, DK], BF16, tag="w", name=name)
        srcw = wap.rearrange("h (dt p) k -> p dt h k", p=P)
        for dt_ in range(DT):
            nc.gpsimd.dma_start(wt[:, dt_], srcw[:, dt_])
        return wt

    # ---- inputs: stage bf16, transpose-load to [d, s] layout ----
    # chunked so each transpose only waits for its own staging rows
    def load_xT(xpool, x_ap, label, first_chunks=None):
        stage = dram.tile([S, D], BF16, tag=f"stage_{label}")
        n_chunks = max(1, S // 512)
        rows = S // n_chunks
        xT = xpool.tile([P, DT, S], BF16, tag="xT", name=f"{label}T")

        def emit(c):
            sl = slice(c * rows, (c + 1) * rows)
            nc.gpsimd.dma_start(stage[sl, :], x_ap[sl, :])
            # one 3D xbar transpose per chunk: xT[p, dt, s] = stage[s, dt*128+p]
            nc.sync.dma_start_transpose(xT[:, :, sl], stage[sl, :])

        if first_chunks is None:
            for c in range(n_chunks):
                emit(c)
            return xT
        for c in range(first_chunks):
            emit(c)

        def finish():
            for c in range(first_chunks, n_chunks):
                emit(c)

        return xT, finish

    # ---- projections (PSUM tiles on the "av" tag so the attention-score
    # pipeline's "sc" slots are free from the start) ----
    kproj = res.tile([P, NPAIR, S], BF16, tag="kproj")
    qproj = res.tile([P, NPAIR, S], BF16, tag="qproj")
    vaug = res.tile([P, H, TT, DK + 1], BF16, tag="vaug")
    nc.vector.memset(vaug[:, :, :, DK : DK + 1], 1.0)

    ones_sb = consts.tile([1, DK], BF16, tag="ones")
    nc.vector.memset(ones_sb[:], 1.0)

    def project_T(xT, w, dst):
        # dst[h2*64+dk, pair, s] = sum_d w[d, pair, h2, dk] * xT[d, s]
        # scq outer: group (scq, pr) only needs transpose chunk scq, so the
        # PE consumption rate matches the staging+transpose feed rate
        for scq in range(S // W5):
            for pr in range(NPAIR):
                ps = ps_av.tile([P, W5], F32, tag="av", name="proj_ps")
                for dt_ in range(DT):
                    nc.tensor.matmul(
                        ps,
                        w[:, dt_, pr],
                        xT[:, dt_, scq * W5 : (scq + 1) * W5],
                        start=dt_ == 0,
                        stop=dt_ == DT - 1,
                    )
                nc.vector.tensor_copy(
                    out=dst[:, pr, scq * W5 : (scq + 1) * W5], in_=ps
                )

    with tc.tile_pool(name="xpool", bufs=2) as xpool:
        kT, finish_k = load_xT(xpool, k_ap, "k", first_chunks=1)
        wk = load_w("wk", wk_ap)
        finish_k()
        project_T(kT, wk, kproj)
        wv = load_w("wv", wv_ap)

        # v projected natural [t, h*dk]; half-major so heads 0-7 finish first
        vT = load_xT(xpool, v_ap, "v")
        for half in range(2):
            for tt in range(TT):
                ps = ps_av.tile([P, 512], F32, tag="av", name="vproj_ps")
                for dt_ in range(DT):
                    nc.tensor.matmul(
                        ps,
                        vT[:, dt_, tt * P : (tt + 1) * P],
                        wv[:, dt_, half * 4 : (half + 1) * 4],
                        start=dt_ == 0,
                        stop=dt_ == DT - 1,
                    )
                nc.vector.tensor_copy(
                    out=vaug[:, half * 8 : (half + 1) * 8, tt, 0:DK],
                    in_=ps.rearrange("p (h k) -> p h k", k=DK),
                )

        # q last, pair-major: attention for pair 0 unblocks as soon as its
        # q slice is projected, overlapping the rest of q-proj with attention
        wq = load_w("wq", wq_ap)
        qT = load_xT(xpool, q_ap, "q")
        project_T(qT, wq, qproj)

    # xpool released; woT lives in the reclaimed space (needed only once the
    # first s-chunk finishes)
    res2 = ctx.enter_context(tc.tile_pool(name="res2", bufs=1))
    woT = res2.tile([P, DT, D], BF16, tag="woT")
    wo_stage = dram.tile([D, D], BF16, tag="wo_stage")
    nc.gpsimd.dma_start(wo_stage[:], wo_ap)
    nc.sync.dma_start_transpose(woT[:], wo_stage[:])

    apool = ctx.enter_context(tc.tile_pool(name="apool", bufs=14))
    spool = ctx.enter_context(tc.tile_pool(name="spool", bufs=1))
    fpool = ctx.enter_context(tc.tile_pool(name="fpool", bufs=3))

    # ---- attention; the previous chunk's Wo projection is interleaved into
    # the pair loop so its PSUM/PE use rides along without starving ScalarE ----
    exp_f = mybir.ActivationFunctionType.Exp

    def final_proj_step(outT_prev, sc_prev, st):
        # returns throttle-point matmul instructions; the caller may pin them
        # behind later exps so the PE never runs a long final-proj block while
        # ScalarE waits on scores
        s0p = sc_prev * SCW
        throttle = []
        for dc in range(D // W5):
            f_ps = ps_av.tile([P, W5], F32, tag="av", name="f_ps")
            for kt in range(DT):
                mm = nc.tensor.matmul(
                    f_ps,
                    outT_prev[:, kt, st * P : (st + 1) * P],
                    woT[:, kt, dc * W5 : (dc + 1) * W5],
                    start=kt == 0,
                    stop=kt == DT - 1,
                )
                if kt in (0, DT // 2):
                    throttle.append(mm)
            fo = fpool.tile([P, W5], F32, tag="fo")
            nc.vector.tensor_copy(out=fo[:], in_=f_ps[:])
            nc.sync.dma_start(
                out_ap[s0p + st * P : s0p + (st + 1) * P, dc * W5 : (dc + 1) * W5],
                fo[:],
            )
        return throttle

    def emit_scores(sc_, pr, tt):
        s0 = sc_ * SCW
        sc_ps = [
            ps_sc.tile([P, SCW], F32, tag="sc", name=f"sc{h2}") for h2 in range(2)
        ]
        for h2 in range(2):
            rows = slice(h2 * DK, (h2 + 1) * DK)
            lhsT = kproj[rows, pr, tt * P : (tt + 1) * P]
            for sh in range(NH):
                nc.tensor.matmul(
                    sc_ps[h2][:, sh * W5 : (sh + 1) * W5],
                    lhsT,
                    qproj[rows, pr, s0 + sh * W5 : s0 + (sh + 1) * W5],
                )
        return sc_ps

    outT_prev = None
    outT = None
    fps_pending = []
    st_per_pair = max(1, (SCW // P) // NPAIR)  # final-proj subtiles per pair
    units = [(sc_, pr) for sc_ in range(SC) for pr in range(NPAIR)]
    for ui, (sc_, pr) in enumerate(units):
        if pr == 0:
            outT_prev = outT
            outT = wpool.tile([P, NPAIR, SCW], BF16, tag="w", name="outT")
        sc_ps = emit_scores(sc_, pr, 0)
        fps_sched = {}  # dep-pinning of woven final-proj: disabled (scheduled worse)
        av_ps = [
            ps_av.tile([DK + 1, SCW], F32, tag="av", name=f"av{h2}")
            for h2 in range(2)
        ]
        # software-pipelined: the next exp's scores (including the next
        # pair's first t-tile) are always emitted before AV / normalize /
        # final-proj matmuls, so ScalarE's next input is never queued
        # behind them on the PE
        for tt in range(TT):
            ats = []
            for h2 in range(2):
                at = apool.tile([P, SCW], BF16, tag="attn", name="at")
                ei = nc.scalar.activation(at[:], sc_ps[h2][:], exp_f, scale=scale)
                ats.append(at)
                if h2 == 0 and tt in fps_sched and fps_sched[tt] < len(fps_pending):
                    add_dep_helper(
                        fps_pending[fps_sched[tt]].ins,
                        ei.ins,
                        sync=True,
                        reason="spread woven final-proj into attention slack",
                    )
            if tt + 1 < TT:
                sc_ps = emit_scores(sc_, pr, tt + 1)
            for h2 in range(2):
                va = vaug[:, 2 * pr + h2, tt, :]
                for sh in range(NH):
                    nc.tensor.matmul(
                        av_ps[h2][:, sh * W5 : (sh + 1) * W5],
                        va,
                        ats[h2][:, sh * W5 : (sh + 1) * W5],
                        start=tt == 0,
                        stop=tt == TT - 1,
                    )
        # normalize: out_hT = av[0:64] * (1 / av[64]) broadcast over rows
        for h2 in range(2):
            rec = spool.tile([1, SCW], F32, tag="rec")
            nc.vector.reciprocal(rec[:], av_ps[h2][DK : DK + 1, :])
            recb = spool.tile([1, SCW], BF16, tag="recb")
            nc.vector.tensor_copy(out=recb[:], in_=rec[:])
            bc_sb = spool.tile([DK, SCW], BF16, tag="bc_sb")
            nc.gpsimd.partition_broadcast(bc_sb[:], recb[:])
            nc.vector.tensor_tensor(
                outT[h2 * DK : (h2 + 1) * DK, pr, :],
                av_ps[h2][0:DK, :],
                bc_sb[:],
                mybir.AluOpType.mult,
            )
        # weave the previous chunk's output projection into this pair loop;
        # its throttle points get pinned behind the NEXT pair's exps
        fps_pending = []
        if outT_prev is not None:
            for i in range(st_per_pair):
                st = pr * st_per_pair + i
                if st < SCW // P:
                    fps_pending = final_proj_step(outT_prev, sc_ - 1, st)

    for st in range(SCW // P):
        final_proj_step(outT, SC - 1, st)


@functools.lru_cache(maxsize=2)
def build(S: int = S_FULL):
    nc = bacc.Bacc("TRN2", target_bir_lowering=False, debug=False)
    with tile.TileContext(nc) as tc:
        with ExitStack() as ctx:
            _body(ctx, tc, S)
    nc.compile()
    return nc


def kernel(**inputs: np.ndarray) -> np.ndarray:
    query = np.ascontiguousarray(inputs["query"], dtype=np.float32)
    key = np.ascontiguousarray(inputs["key"], dtype=np.float32)
    value = np.ascontiguousarray(inputs["value"], dtype=np.float32)
    Wq = np.ascontiguousarray(inputs["Wq"], dtype=np.float32)
    Wk = np.ascontiguousarray(inputs["Wk"], dtype=np.float32)
    Wv = np.ascontiguousarray(inputs["Wv"], dtype=np.float32)
    Wo = np.ascontiguousarray(inputs["Wo"], dtype=np.float32)

    nc = build(S_FULL)
    in_maps = [
        {
            "q": query[i],
            "k": key[i],
            "v": value[i],
            "Wq": Wq,
            "Wk": Wk,
            "Wv": Wv,
            "Wo": Wo,
        }
        for i in range(N_CORES)
    ]
    res = run_bass_kernel_spmd(nc, in_maps, core_ids=list(range(N_CORES)))
    return np.stack([res.results[i]["out"] for i in range(N_CORES)], axis=0)


if __name__ == "__main__":
    rng = np.random.default_rng(0)
    ins = {
        "query": rng.standard_normal((B, S_FULL, D), dtype=np.float32),
        "key": rng.standard_normal((B, S_FULL, D), dtype=np.float32),
        "value": rng.standard_normal((B, S_FULL, D), dtype=np.float32),
        "Wq": rng.standard_normal((H, D, DK), dtype=np.float32) * 0.02,
        "Wk": rng.standard_normal((H, D, DK), dtype=np.float32) * 0.02,
        "Wv": rng.standard_normal((H, D, DK), dtype=np.float32) * 0.02,
        "Wo": rng.standard_normal((D, D), dtype=np.float32) * 0.02,
    }
    out = kernel(**ins)
    print(out.shape, out.dtype)
